# revision 39
# baseline (speedup 1.0000x reference)
"""Multi-head attention (B=4, S=2048, D=1024, H=16) on 8 TRN2 NeuronCores.

Sharding: data-parallel over batch (4) x tensor-parallel over head halves (2).
Core c handles batch b = c//2 and heads [8g, 8g+8) where g = c%2. Each core
computes a partial [S, D] output-projection contribution; the host sums the
two head-group partials per batch (plus an "out2" partial for the split
q-chunk, see below).

Both the scalar engine's exp() stream (256 x [128,1024] activations,
~294us busy) and the PE stream (~246us of matmul columns + switch bubbles)
are near the span floor, so the schedule keeps BOTH dense:

  - 256 global steps grouped in 128 2-step groups. Group g (s=2g) emits:
    [proj fill ~1 unit] score(s+1) score(s+2) pv(s-1) pv(s) dquad(s-1)
    on the PE and exp(s), exp(s+1) on the scalar engine. Scores lead
    their exp by 1-2 steps (sc PSUM bufs=2); PV lags by 1-2 (P bufs=4).
  - Window order is boustrophedon in (qc, hpp):
    (0,0)(1,0)(2,0)(3,0)(3,1)(2,1)(1,1)(0,1). This spreads the K/V/Q
    projection deadlines over the WHOLE run (K t01 + V half0 + Q t01 in
    the first four windows; K t23 + V half1 + Q t23 in the last four)
    instead of cramming them into windows 0-1, and staggers when each
    q-chunk's w_o projection becomes runnable (qc3 at g82, qc2 at g98,
    qc1 at g114, qc0 split).
  - Projection work is chopped into ~850ns units (4 matmuls) scheduled
    EDF with a guaranteed-1-unit-per-group floor; units interleave with
    the attention slots so consecutive units don't stall on the single
    pacc bank.
  - qc0's w_o is split by head-pair: hp 0,1 stream mid-run into a
    separate DRAM partial "out2" (host adds it back); hp 2,3 run in the
    short drain after the last exp.
  - Outputs are bf16 (host upcasts + sums) to halve the tail DMA.

PSUM budget (8 banks): 2 x sc [128,1024] (4) + 2 x U [128,512] (2) +
d [128,512] (1) + proj accumulator (1).
"""

import heapq

import numpy as np

B, S, D, H = 4, 2048, 1024, 16
DK = D // H          # 64
G = 2                # head groups (tensor-parallel degree per batch)
HL = H // G          # 8 local heads per core
DV = HL * DK         # 512 local value dim
N_CORES = 8
NQC = 4              # q-chunks of 512
NKT = 16             # k-tiles of 128

# window order: (qc, hpp) per 32-step window
WINS = [(0, 0), (1, 0), (2, 0), (3, 0), (3, 1), (2, 1), (1, 1), (0, 1)]

_cached = {}


def _build():
    import concourse.bass as bass
    import concourse.tile as tile
    from concourse import bacc, mybir

    f32 = mybir.dt.float32
    bf16 = mybir.dt.bfloat16
    EXP = mybir.ActivationFunctionType.Exp

    nc = bacc.Bacc("TRN2", target_bir_lowering=False, debug=False,
                   num_devices=N_CORES)

    # All host-packed so big DMAs get 8KB-contiguous rows.
    wkW = nc.dram_tensor("wkW", [128, 4096], bf16, kind="ExternalInput").ap()
    wqW = nc.dram_tensor("wqW", [128, 4096], bf16, kind="ExternalInput").ap()
    wvW = nc.dram_tensor("wvW", [128, 4096], bf16, kind="ExternalInput").ap()
    woW = nc.dram_tensor("woW", [128, 4096], bf16, kind="ExternalInput").ap()
    xkC = [nc.dram_tensor(f"xk{c}", [128, 4096], bf16,
                          kind="ExternalInput").ap() for c in range(4)]
    xvC = [nc.dram_tensor(f"xv{c}", [128, 4096], bf16,
                          kind="ExternalInput").ap() for c in range(4)]
    xqC = [nc.dram_tensor(f"xq{c}", [128, 4096], bf16,
                          kind="ExternalInput").ap() for c in range(4)]
    out = nc.dram_tensor("out", [S, D], bf16, kind="ExternalOutput").ap()
    out2 = nc.dram_tensor("out2", [512, D], bf16, kind="ExternalOutput").ap()

    with tile.TileContext(nc) as tc:
        with (
            tc.tile_pool(name="persist", bufs=1) as persist,
            tc.tile_pool(name="xpool", bufs=1) as xpool,
            tc.tile_pool(name="ppool", bufs=4) as ppool,
            tc.tile_pool(name="work", bufs=2) as work,
            tc.tile_pool(name="psum", bufs=1,
                         space=bass.MemorySpace.PSUM) as psum,
        ):
            KT = {}      # (t, c) -> [128, 512] bf16 K^T tiles
            QT = {}      # (t, qc) -> [128, 512]
            VT = {}      # kt -> [128, 512] (all 8 heads' V columns)
            OT = {}      # (qc, hp) -> [128, 512] normalized out^T
            Ubank = {}   # parity -> live U psum tile
            Dbank = {}   # live d psum tile
            wsb = {}
            xk_sb, xv_sb, xq_sb = {}, {}, {}

            def step_ids(u):
                qc, hpp = WINS[u // 32]
                loc = u % 32
                return qc, hpp, 2 * hpp + (loc % 2), loc // 2, loc % 2

            # ---------------- DMA staging ----------------
            def load_w(name, src, engine=None, sliced=False):
                t = persist.tile([128, 4096], bf16, tag=f"w_{name}", name="w")
                e = engine or nc.sync
                if sliced:
                    for blk in range(4):
                        e.dma_start(t[:, 1024 * blk:1024 * (blk + 1)],
                                    src[:, 1024 * blk:1024 * (blk + 1)])
                else:
                    e.dma_start(t[:], src[:, :])
                wsb[name] = t

            def load_x(dst, key, src, c, engine, bufs, sliced=False):
                t = xpool.tile([128, 4096], bf16, tag=f"x_{key}", name="x",
                               bufs=bufs)
                if sliced:
                    for d in range(8):
                        engine.dma_start(t[:, 512 * d:512 * (d + 1)],
                                        src[:, 512 * d:512 * (d + 1)])
                else:
                    engine.dma_start(t[:], src[:, :])
                dst[c] = t

            # ---------------- projection pieces (split into units) ------
            def proj_units(wname, xt, t, done):
                # two 4-matmul units accumulating [128,512]; done(acc) last
                cell = {}

                def u1():
                    cell["acc"] = psum.tile([128, 512], f32, tag="pacc",
                                            name="pacc")
                    for d in range(4):
                        nc.tensor.matmul(
                            cell["acc"][:],
                            wsb[wname][:, 1024 * t + 128 * d:
                                       1024 * t + 128 * (d + 1)],
                            xt[:, 512 * d:512 * (d + 1)],
                            start=(d == 0), stop=False)

                def u2():
                    for d in range(4, 8):
                        nc.tensor.matmul(
                            cell["acc"][:],
                            wsb[wname][:, 1024 * t + 128 * d:
                                       1024 * t + 128 * (d + 1)],
                            xt[:, 512 * d:512 * (d + 1)],
                            start=False, stop=(d == 7))
                    done(cell["acc"])
                return [u1, u2]

            def k_units(t, c):
                def done(acc):
                    kt_ = persist.tile([128, 512], bf16, tag=f"kT{t}_{c}",
                                       name="kT")
                    nc.vector.tensor_copy(kt_[:], acc[:])
                    KT[(t, c)] = kt_
                return proj_units("wk", xk_sb[c], t, done)

            def q_units(t, qc):
                def done(acc):
                    qt_ = persist.tile([128, 512], bf16, tag=f"qT{t}",
                                       name="qT", bufs=2)
                    nc.vector.tensor_copy(qt_[:], acc[:])
                    QT[(t, qc)] = qt_
                return proj_units("wq", xq_sb[qc], t, done)

            def v_units(kt, w):
                c, j = kt // 4, kt % 4
                cell = {}

                def u():
                    cell["acc"] = psum.tile([128, 512], f32,
                                            tag="pacc", name="pacc")
                    for d in range(8):
                        nc.tensor.matmul(
                            cell["acc"][:, 0:256],
                            xv_sb[c][:, 512 * d + 128 * j:
                                      512 * d + 128 * (j + 1)],
                            wsb["wv"][:, 512 * d + 256 * w:
                                      512 * d + 256 * (w + 1)],
                            start=(d == 0), stop=(d == 7))
                    if kt not in VT:
                        VT[kt] = persist.tile(
                            [128, 512], bf16, tag=f"v{kt}", name="v")
                    nc.vector.tensor_copy(
                        VT[kt][:, 256 * w:256 * (w + 1)],
                        cell["acc"][:, 0:256])
                return [u]

            def wo_units(qc, st, ncol, acc_tag="pacc", trange=(0, 4),
                         dest=None):
                cell = {}
                tlo0, thi0 = trange
                dst = out if dest is None else dest
                row0 = (512 * qc if dest is None else 0) + 128 * st

                def mk(tlo, thi, first, last):
                    def u():
                        if first:
                            cell["acc"] = psum.tile(
                                [128, 512], f32, tag=acc_tag, name="pacc",
                                bufs=(2 if acc_tag == "sc" else None))
                        for t in range(tlo, thi):
                            nc.tensor.matmul(
                                cell["acc"][:],
                                OT[(qc, t)][:, 128 * st:128 * (st + 1)],
                                wsb["wo"][:, 1024 * t + 512 * ncol:
                                          1024 * t + 512 * (ncol + 1)],
                                start=(t == tlo and first),
                                stop=(t == thi - 1 and last))
                        if last:
                            ob = work.tile([128, 512], bf16, tag="ob",
                                           name="ob", bufs=2)
                            nc.vector.tensor_copy(ob[:], cell["acc"][:])
                            nc.sync.dma_start(
                                dst[row0:row0 + 128,
                                    512 * ncol:512 * (ncol + 1)], ob[:])
                    return u
                if thi0 - tlo0 <= 2:
                    return [mk(tlo0, thi0, True, True)]
                mid = (tlo0 + thi0) // 2
                return [mk(tlo0, mid, True, False), mk(mid, thi0, False, True)]

            # ---------------- attention machinery ----------------
            ones = persist.tile([128, 1], bf16, tag="ones", name="ones")
            nc.vector.tensor_copy(
                ones[:], nc.const_aps.tensor(1.0, (128, 1), bf16))

            P_of = {}    # step -> P tile
            sc_q = {}

            def emit_score(u):
                qc, hpp, hp, kt, par = step_ids(u)
                c, j = kt // 4, kt % 4
                sc = psum.tile([128, 1024], f32, tag="sc", name="sc", bufs=2)
                for i in range(2):
                    po = 64 * i
                    nc.tensor.matmul(
                        sc[:, 512 * i:512 * (i + 1)],
                        KT[(hp, c)][po:po + 64, 128 * j:128 * (j + 1)],
                        QT[(hp, qc)][po:po + 64, :],
                        start=True, stop=True)
                sc_q[u] = sc

            def emit_exp(u):
                p = ppool.tile([128, 1024], bf16, tag="p", name="p")
                nc.scalar.activation(p[:], sc_q.pop(u)[:], EXP, scale=0.125)
                P_of[u] = p

            def emit_pv(u):
                qc, hpp, hp, kt, par = step_ids(u)
                if kt == 0:
                    Ubank[par] = psum.tile([128, 512], f32, tag="u",
                                           name="u", bufs=2)
                U = Ubank[par]
                p = P_of[u]
                for i in range(2):
                    nc.tensor.matmul(
                        U[64 * i:64 * (i + 1), :],
                        VT[kt][:, 128 * hp + 64 * i:128 * hp + 64 * (i + 1)],
                        p[:, 512 * i:512 * (i + 1)],
                        start=(kt == 0), stop=(kt == 15),
                        skip_group_check=True)

            def emit_dquad(u):
                # u odd: P(u-1) = even parity, P(u) = odd parity, same kt
                qc, hpp, hp, kt, par = step_ids(u)
                if kt == 0:
                    Dbank["d"] = psum.tile([128, 512], f32, tag="d", name="d")
                    # zero the unused rows once (mid-window, DVE has slack)
                    # so the single full-tile dsb copy reads no junk
                    nc.vector.memset(Dbank["d"][:], 0.0)
                db = Dbank["d"]
                srcs = [P_of[u - 1][:, 0:512], P_of[u - 1][:, 512:1024],
                        P_of[u][:, 0:512], P_of[u][:, 512:1024]]
                for idx, src in enumerate(srcs):
                    nc.tensor.matmul(
                        db[32 * idx:32 * idx + 1, :], ones[:], src,
                        start=(kt == 0), stop=(kt == 15),
                        tile_position=(0, 32 * idx), skip_group_check=True)
                P_of.pop(u - 1, None)
                P_of.pop(u, None)

            Usb_of = {}

            def emit_ucopy(par):
                # rides the scalar queue: boundaries are exactly where the
                # exp stream has a structural gap, and this frees the DVE
                usb = work.tile([128, 512], f32, tag=f"usb{par}", name="usb",
                                bufs=2)
                nc.scalar.copy(usb[:], Ubank[par][:])
                Usb_of[par] = usb

            def normalize_pieces(w):
                # d -> dsb copy runs NOW (d bank recycles next group); the
                # recip/broadcast/mul chain is returned as 4 deferred
                # pieces (one per group) so the in-order DVE queue never
                # buries the pacc->SBUF copies the PE is waiting on.
                qc, hpp = WINS[w]
                dsb = work.tile([128, 512], f32, tag="dsb", name="dsb",
                                bufs=2)
                nc.scalar.copy(dsb[:], Dbank["d"][:])
                cell = {}

                def piece_a(par, i):
                    # dr-DMA (row -> partition 0), recip, broadcast; the
                    # matching mul runs a group LATER so the DVE never
                    # blocks in-queue on the slow gpsimd broadcast.
                    row = 64 * par + 32 * i
                    dr = work.tile([1, 512], f32, tag="dr", name="dr",
                                   bufs=2)
                    nc.sync.dma_start(dr[:], dsb[row:row + 1, :])
                    rr = work.tile([1, 512], f32, tag="rr", name="rr",
                                   bufs=2)
                    nc.vector.reciprocal_approx_fast(rr[:], dr[:])
                    if i == 0:
                        rb = work.tile([64, 512], f32, tag="rb0",
                                       name="rb", bufs=1)
                    else:
                        rb = work.tile([128, 512], f32, tag="rb1",
                                       name="rb", bufs=1)
                    nc.gpsimd.partition_broadcast(rb[:], rr[:])
                    cell[(par, i)] = rb

                def piece_b(par, i, ot, usb):
                    rb = cell.pop((par, i))
                    if i == 0:
                        nc.vector.tensor_mul(ot[0:64, :], usb[0:64, :],
                                             rb[:])
                    else:
                        nc.vector.tensor_mul(ot[64:128, :], usb[64:128, :],
                                             rb[64:128, :])

                ab = []
                for par in range(2):
                    hp = 2 * hpp + par
                    ot = persist.tile([128, 512], bf16, tag=f"oT{qc}_{hp}",
                                      name="oT")
                    OT[(qc, hp)] = ot
                    usb = Usb_of[par]
                    for i in range(2):
                        ab.append((
                            lambda par=par, i=i: piece_a(par, i),
                            lambda par=par, i=i, ot=ot, usb=usb:
                            piece_b(par, i, ot, usb)))

                def seq2(*fns):
                    def fn():
                        for f in fns:
                            f()
                    return fn
                return [ab[0][0],
                        ab[1][0],
                        seq2(ab[2][0], ab[0][1]),
                        seq2(ab[3][0], ab[1][1]),
                        ab[2][1],
                        ab[3][1]]

            def emit_normalize(w):
                for p in normalize_pieces(w):
                    p()

            def emit_normalize_drain(w):
                # drain-only fast path: no dr-DMA hop and no gpsimd —
                # recip runs lane-wise at the (32-aligned) source row and
                # the idle PE broadcasts it (K=1 ones-matmul) into the
                # now-free d bank; DVE muls read rb straight from PSUM.
                qc, hpp = WINS[w]
                dsb = work.tile([128, 512], f32, tag="dsb", name="dsb",
                                bufs=2)
                nc.vector.tensor_copy(dsb[:], Dbank["d"][:])
                rbp = psum.tile([128, 512], f32, tag="d", name="d")
                ones32 = persist.tile([128, 64], f32, tag="ones32",
                                      name="ones32")
                nc.vector.tensor_copy(
                    ones32[:], nc.const_aps.tensor(1.0, (128, 64), f32))
                for par in range(2):
                    hp = 2 * hpp + par
                    ot = persist.tile([128, 512], bf16, tag=f"oT{qc}_{hp}",
                                      name="oT")
                    OT[(qc, hp)] = ot
                    usb = Usb_of[par]
                    for i in range(2):
                        row = 64 * par + 32 * i
                        rr = work.tile([128, 512], f32, tag="rrd",
                                       name="rr", bufs=2)
                        nc.vector.reciprocal_approx_fast(
                            rr[row:row + 1, :], dsb[row:row + 1, :])
                        nc.tensor.matmul(
                            rbp[64 * i:64 * (i + 1), :],
                            ones32[row:row + 1, :],
                            rr[row:row + 1, :],
                            start=True, stop=True,
                            tile_position=(row, 64 * i),
                            skip_group_check=True)
                        nc.vector.tensor_mul(
                            ot[64 * i:64 * (i + 1), :],
                            usb[64 * i:64 * (i + 1), :],
                            rbp[64 * i:64 * (i + 1), :])

            # ---------------- emission schedule ----------------
            # scalar-engine ACT table preload (runs during DMA prologue)
            jin = work.tile([128, 8], f32, tag="jin", name="jin", bufs=1)
            nc.vector.tensor_copy(jin[:],
                                  nc.const_aps.tensor(0.0, (128, 8), f32))
            jout = work.tile([128, 8], bf16, tag="jout", name="jout", bufs=1)
            nc.scalar.activation(jout[:], jin[:], EXP)

            # PE p-state warmup: the tensor engine only reaches max clock
            # after ~3us of continuous work. Dummy matmuls (no DMA deps)
            # keep it busy through the DMA prologue so the first real
            # projections run at full speed. Output goes to the d-tagged
            # bank (first real use is ~30us in).
            wconst = persist.tile([128, 512], bf16, tag="wconst",
                                  name="wconst")
            nc.vector.tensor_copy(
                wconst[:], nc.const_aps.tensor(1.0, (128, 512), bf16))

            def pe_warm(n, tag, bufs=None):
                wps = psum.tile([128, 512], f32, tag=tag, name=tag,
                                bufs=bufs)
                for _ in range(n):
                    nc.tensor.matmul(wps[0:1, :], ones[:], wconst[:],
                                     start=True, stop=True,
                                     skip_group_check=True)

            pe_warm(24, "d")

            # DMA plan: the scalar queue must stay CLEAN of bulk DMA
            # descriptors (each issue occupies the queue for the transfer
            # time, and the exp stream lives there) — it only gets the two
            # small Q-side criticals, which finish issuing before exp(0)
            # can run anyway. Everything else rides sync, ordered by
            # first-consumer time.
            wk_t = persist.tile([128, 4096], bf16, tag="w_wk", name="w")
            wsb["wk"] = wk_t
            wq_t = persist.tile([128, 4096], bf16, tag="w_wq", name="w")
            wsb["wq"] = wq_t
            nc.scalar.dma_start(wq_t[:, 0:1024], wqW[:, 0:1024])
            load_x(xq_sb, "q", xqC[0], 0, nc.scalar, 2)
            nc.sync.dma_start(wk_t[:, 0:1024], wkW[:, 0:1024])
            xk0_t = xpool.tile([128, 4096], bf16, tag="x_k", name="x",
                               bufs=4)
            nc.sync.dma_start(xk0_t[:, 0:2048], xkC[0][:, 0:2048])
            nc.sync.dma_start(xk0_t[:, 2048:4096], xkC[0][:, 2048:4096])
            xk_sb[0] = xk0_t
            nc.sync.dma_start(wk_t[:, 1024:2048], wkW[:, 1024:2048])
            nc.sync.dma_start(wq_t[:, 1024:2048], wqW[:, 1024:2048])
            load_w("wv", wvW)
            load_x(xv_sb, "v", xvC[0], 0, nc.sync, 4)
            load_x(xk_sb, "k", xkC[1], 1, nc.sync, 4)
            load_x(xv_sb, "v", xvC[1], 1, nc.sync, 4)
            load_x(xk_sb, "k", xkC[2], 2, nc.sync, 4)
            load_x(xv_sb, "v", xvC[2], 2, nc.sync, 4)
            load_x(xk_sb, "k", xkC[3], 3, nc.sync, 4)
            load_x(xv_sb, "v", xvC[3], 3, nc.sync, 4)
            load_x(xq_sb, "q", xqC[1], 1, nc.sync, 2)
            nc.sync.dma_start(wk_t[:, 2048:4096], wkW[:, 2048:4096])
            nc.sync.dma_start(wq_t[:, 2048:4096], wqW[:, 2048:4096])
            load_w("wo", woW)

            # unit heap: (deadline_group, seq, fn). Each entry is one
            # SELF-CONTAINED pacc lifetime (~850ns of PE work) so entries
            # can never interleave inside another's PSUM accumulation.
            pieces = []
            seq = [0]

            def whole(units):
                def fn():
                    for u in units:
                        u()
                return fn

            pending = []   # (avail_group, dl, fn): not poppable before avail

            def push(dl, fn, avail=None):
                if avail is not None:
                    pending.append((avail, dl, fn))
                    return
                heapq.heappush(pieces, (dl, seq[0], fn))
                seq[0] += 1

            # K: t01 used in windows 0-3 (first use window 0, step 8c+t%2);
            #    t23 used in windows 4-7 (first use window 4).
            for t in range(4):
                for c in range(4):
                    if (t, c) in ((0, 0), (1, 0)):
                        continue
                    u0 = 32 * (0 if t < 2 else 4) + 8 * c + (t % 2)
                    push((u0 - 1) // 2 - 3, whole(k_units(t, c)))
            # Q t01 for qc=1 (xq1 lands in the prologue); the rest are
            # pushed when their xq chunk (re)loads, below.
            for t in range(2):
                u0 = 32 * 1 + t
                push((u0 - 1) // 2 - 3, whole(q_units(t, 1)))
            # V: half 0 used by hpp=0 windows (0-3), half 1 by windows 4-7.
            for w in range(2):
                for kt in range(NKT):
                    if w == 0 and kt == 0:
                        continue
                    push(64 * w + kt - 1, whole(v_units(kt, w)))

            def collect(g):
                for ent in [e for e in pending if e[0] <= g]:
                    pending.remove(ent)
                    heapq.heappush(pieces, (ent[1], seq[0], ent[2]))
                    seq[0] += 1
                got, fns = 0, []
                while (pieces and got < 3
                       and (pieces[0][0] <= g or got < 1)):
                    _, _, fn = heapq.heappop(pieces)
                    fns.append(fn)
                    got += 1
                return fns

            # prologue compute: step 0 and deps for step 1 (window (0,0))
            for un in k_units(0, 0) + q_units(0, 0):
                un()
            emit_score(0)
            for un in k_units(1, 0) + q_units(1, 0):
                un()
            push(-1, whole(v_units(0, 0)))

            def win_done(wdone, g):
                # (qc, hpp=1) windows completing a q-chunk: stream its w_o
                # (deadlines leave room for the deferred normalize pieces)
                qcd, hppd = WINS[wdone]
                if wdone >= 4:
                    step = 1 if wdone == 6 else 2
                    for i2, (st, ncol) in enumerate(
                            (st, ncol) for st in range(4)
                            for ncol in range(2)):
                        push(g + 9 + step * i2,
                             whole(wo_units(qcd, st, ncol)), avail=g + 8)
                elif wdone == 0:
                    # qc0 hp01 partial -> out2 (host adds it back)
                    for i2, (st, ncol) in enumerate(
                            (st, ncol) for st in range(4)
                            for ncol in range(2)):
                        push(g + 10 + 5 * i2,
                             whole(wo_units(0, st, ncol, trange=(0, 2),
                                            dest=out2)), avail=g + 8)

            deferred = []
            NSTEP = 256
            for g in range(NSTEP // 2):
                s = 2 * g
                fills = collect(g)
                # in the first groups, input DMAs are still landing: fills
                # would block the PE queue AHEAD of the scores and stall
                # the exp stream, so run them after score(s+2) instead
                pre = fills if g >= 3 else []
                post = [] if g >= 3 else fills
                emit_exp(s)
                if pre[0:1]:
                    pre[0]()
                if s + 1 <= NSTEP - 1:
                    emit_score(s + 1)
                if pre[1:2]:
                    pre[1]()
                if s + 2 <= NSTEP - 1:
                    emit_score(s + 2)
                if pre[2:3]:
                    pre[2]()
                for fn in post:
                    fn()
                if s + 1 <= NSTEP - 1:
                    emit_exp(s + 1)
                if s - 1 >= 0:
                    emit_pv(s - 1)
                    if (s - 1) % 32 == 31:
                        emit_ucopy(1)
                emit_pv(s)
                if s % 32 == 30:
                    emit_ucopy(0)
                if s - 1 >= 1:
                    emit_dquad(s - 1)
                    if (s - 1) % 32 == 31:
                        deferred.extend(normalize_pieces((s - 1) // 32))
                        win_done((s - 1) // 32, g)
                for fn in (fills[3:] if g >= 3 else []):
                    fn()
                if deferred:
                    deferred.pop(0)()
                # xq chunk (re)loads (bufs=2 rotation), with the q units
                # that depend on them pushed only once the load is issued
                if g == 8:
                    load_x(xq_sb, "q", xqC[2], 2, nc.sync, 2)
                    push(26, whole(q_units(0, 2)))
                    push(27, whole(q_units(1, 2)))
                    push(76, whole(q_units(2, 2)))
                    push(77, whole(q_units(3, 2)))
                elif g == 24:
                    load_x(xq_sb, "q", xqC[3], 3, nc.sync, 2)
                    push(42, whole(q_units(0, 3)))
                    push(43, whole(q_units(1, 3)))
                    push(58, whole(q_units(2, 3)))
                    push(59, whole(q_units(3, 3)))
                elif g == 80:
                    load_x(xq_sb, "q", xqC[1], 1, nc.sync, 2)
                    push(90, whole(q_units(2, 1)))
                    push(91, whole(q_units(3, 1)))
                elif g == 96:
                    load_x(xq_sb, "q", xqC[0], 0, nc.sync, 2)
                    push(106, whole(q_units(2, 0)))
                    push(107, whole(q_units(3, 0)))

            # drain: last pv/dquad, final normalize (window 7 = (0,1)),
            # then qc0's hp23 w_o with alternating accumulator tags
            emit_pv(NSTEP - 1)
            emit_ucopy(1)
            emit_dquad(NSTEP - 1)
            # keep the PE clock at max through the final normalize chain
            # so the drain's w_o matmuls run at full speed
            pe_warm(24, "u", bufs=2)
            emit_normalize(7)
            for av, dl, fn in sorted(pending, key=lambda e: e[1]):
                heapq.heappush(pieces, (dl, seq[0], fn))
                seq[0] += 1
            pending.clear()
            while pieces:
                _, _, fn = heapq.heappop(pieces)
                fn()
            for st in range(4):
                for ncol in range(2):
                    tag = "pacc" if (2 * st + ncol) % 2 == 0 else "sc"
                    for un in wo_units(0, st, ncol, acc_tag=tag,
                                       trange=(2, 4)):
                        un()

    nc.compile()
    return nc


def make_in_maps(query, key, value, w_q, w_k, w_v, w_o):
    import ml_dtypes
    bf = ml_dtypes.bfloat16

    def c(a):
        return np.ascontiguousarray(a).astype(bf)

    def pack_w(wT, blocks, width):
        # [blocks*128, width] -> [128, blocks*width] (d-tiles side by side)
        return c(wT.reshape(blocks, 128, width).transpose(1, 0, 2)
                 .reshape(128, blocks * width))

    def pack_tmaj(wT):
        # [1024(d), 512(dk)] -> [128, 4096], col = 1024*t + 128*d + dk_local
        return c(wT.reshape(8, 128, 4, 128).transpose(1, 2, 0, 3)
                 .reshape(128, 4096))

    def pack_x(xT):
        # xT [D, S] -> per k/q-chunk [128, 4096] (8 d-tiles side by side)
        outs = []
        for ch in range(4):
            sl = xT[:, 512 * ch:512 * (ch + 1)]           # [1024, 512]
            outs.append(pack_w(sl, 8, 512))
        return outs

    in_maps = []
    for core in range(N_CORES):
        b, g = core // G, core % G
        rows = slice(DV * g, DV * (g + 1))
        xq = pack_x(np.asarray(query[b], np.float32).T)
        xk = pack_x(np.asarray(key[b], np.float32).T)
        xv = pack_x(np.asarray(value[b], np.float32).T)
        m = {
            "wqW": pack_tmaj(np.asarray(w_q[rows, :], np.float32).T),
            "wkW": pack_tmaj(np.asarray(w_k[rows, :], np.float32).T),
            "wvW": pack_w(np.asarray(w_v[rows, :], np.float32).T, 8, 512),
            "woW": pack_w(np.asarray(w_o[:, rows], np.float32).T, 4, 1024),
        }
        for ch in range(4):
            m[f"xq{ch}"] = xq[ch]
            m[f"xk{ch}"] = xk[ch]
            m[f"xv{ch}"] = xv[ch]
        in_maps.append(m)
    return in_maps


def kernel(query, key, value, w_q, w_k, w_v, w_o):
    from concourse.bass_utils import run_bass_kernel_spmd

    if "nc" not in _cached:
        _cached["nc"] = _build()
    nc = _cached["nc"]

    in_maps = make_in_maps(query, key, value, w_q, w_k, w_v, w_o)
    res = run_bass_kernel_spmd(nc, in_maps, list(range(N_CORES)))
    full = np.empty((B, S, D), np.float32)
    for b in range(B):
        r0, r1 = res.results[G * b], res.results[G * b + 1]
        full[b] = (r0["out"].astype(np.float32)
                   + r1["out"].astype(np.float32))
        # qc0 rows carry only hp23 in "out"; hp01 partial lives in out2
        full[b][0:512] += (r0["out2"].astype(np.float32)
                           + r1["out2"].astype(np.float32))
    return full
